# revision 19
# baseline (speedup 1.0000x reference)
"""Trainium2 Bass kernel for a 12-layer attention-only decoder.

Model (see harness reference): S=24, B=256, D=1024, H=16 heads (dh=64),
L=12 layers, V=32000 vocab.  Per layer: q/k/v projections, softmax
attention (scale 1/sqrt(D), no mask applied), residual add.  Final vocab
head x @ out_w.T + out_b.

Sharding: data-parallel over batch - 8 cores x 32 batches each.
Embedding gather + positional-encoding add is done on host (0.006% of
FLOPs); everything else runs on device with fp32 accumulation.

Precision: Q/K projections run as double-fp8 (e4m3 weights x e4m3
activations, DoubleRow perf mode, 2x PE throughput) - softmax washes the
quantization noise out of the scores entirely (measured end-to-end rel
err identical to bf16).  V projection and the vocab head stay bf16 (v
errors hit the output linearly; fp8 there measures 6e-2 rel err).

On-device layout (per core, 768 tokens = 32 batches x 24 positions):
  - residual x kept feature-major: xT[d, t] as 8 chunk tiles [128, 768]
  - q/k projections produce feature-major qT/kT (stationary = w^T chunk)
  - v projection produces token-major v (stationary = xT chunk), padded
    so each batch sits at a 32-aligned partition base (24 rows used + 8
    pad) -> attention matmul operands satisfy the 32/64-alignment rules
  - scores^T[s', s] = matmul(lhsT=kT slice [64,24], rhs=qT slice [64,24])
  - exp via ScalarE (scale 1/32 folded into q), Z via ones-matmul issued
    one batch-group behind scores (keeps the in-order tensor queue from
    stalling on ScalarE), normalize on VectorE, AV: matmul(lhsT=v slice
    [24,64], rhs=attn^T [24,24]) -> o^T feature-major, accumulated
    straight into xT
  - vb folded into the residual after AV (exact: softmax rows sum to 1)
  - vocab head token-major: psum [128 tokens, 512 vocab] tiles, DMA out.
"""

import math

import numpy as np
import ml_dtypes

import concourse.bass as bass
import concourse.mybir as mybir
import concourse.tile as tile
from concourse import bacc
from concourse.bass_utils import run_bass_kernel_spmd

S, B, D, H, L, V = 24, 256, 1024, 16, 12, 32000
DH = D // H  # 64
NCORES = 8
BL = B // NCORES          # 32 local batches
T = BL * S                # 768 local tokens
KO = D // 128             # 8 contraction chunks
SCALE = 1.0 / math.sqrt(D)
WS = 256.0                # fp8 weight scale (power of 2; undone post-matmul)

F32 = mybir.dt.float32
BF16 = mybir.dt.bfloat16
FP8 = mybir.dt.float8e4

_CACHE = {}


def _build_kernel(n_layers=L, do_attn=True, do_head=True):
    nc = bacc.Bacc(None, target_bir_lowering=False)

    x0t_d = nc.dram_tensor("x0t", [D, T], F32, kind="ExternalInput")
    qwt_d = nc.dram_tensor("qwt", [L, D, D], FP8, kind="ExternalInput")
    kwt_d = nc.dram_tensor("kwt", [L, D, D], FP8, kind="ExternalInput")
    vwt_d = nc.dram_tensor("vwt", [L, D, D], BF16, kind="ExternalInput")
    qbs_d = nc.dram_tensor("qbs", [L, D], F32, kind="ExternalInput")
    kb_d = nc.dram_tensor("kb", [L, D], F32, kind="ExternalInput")
    vbb_d = nc.dram_tensor("vbb", [L, D], F32, kind="ExternalInput")
    owt_d = nc.dram_tensor("owt", [D, V], BF16, kind="ExternalInput")
    out_d = nc.dram_tensor("out", [T, V], F32, kind="ExternalOutput")

    Ident = mybir.ActivationFunctionType.Identity
    Exp = mybir.ActivationFunctionType.Exp
    Add = mybir.AluOpType.add
    Mult = mybir.AluOpType.mult
    DR = mybir.MatmulPerfMode.DoubleRow

    with tile.TileContext(nc) as tc:
        # zero all PSUM once: stale device PSUM may hold inf/NaN, which would
        # poison the block-diag Z matmul via 0*inf
        with tc.tile_pool(name="psinit", bufs=1, space="PSUM") as psi:
            for i in range(8):
                zb = psi.tile([128, 512], F32, name=f"zb_{i}", tag=f"zb_{i}")
                nc.vector.memset(zb[:], 0.0)

        with (
            tc.tile_pool(name="persist", bufs=1) as persist,
            tc.tile_pool(name="psA", bufs=3, space="PSUM") as psA,   # proj/head [128,512]
        ):
            # ---- persistent SBUF state ----
            # per-ko-chunk tiles: dependency tracking is tile-granular, so
            # separate tiles let layer-0 matmuls start on chunk 0 while the
            # rest of x0 is still in flight
            xts = [persist.tile([128, T], F32, name=f"xt{k}") for k in range(KO)]
            xbfs = [persist.tile([128, T], BF16, name=f"xbf{k}") for k in range(KO)]
            xpads = [persist.tile([128, BL * 32], BF16, name=f"xp{k}") for k in range(KO)]
            # fp8 x for the DoubleRow q/k projections, ko-pair layout
            x8p = [persist.tile([128, 2, T], FP8, name=f"x8p{j}") for j in range(KO // 2)]
            qb_sb = persist.tile([128, L, KO], F32, name="qb_sb")
            kb_sb = persist.tile([128, L, KO], F32, name="kb_sb")
            vb_sb = persist.tile([128, L, KO], F32, name="vb_sb")
            onesblk = persist.tile([128, 120], BF16, name="onesblk")

            x0_view = x0t_d[:].rearrange("(ko p) t -> p ko t", p=128)
            nc.vector.memset(onesblk[:], 0.0)
            for bi in range(4):
                nc.vector.memset(onesblk[bi * 32:bi * 32 + S, bi * 32:bi * 32 + S], 1.0)

            def recast():
                # xbf <- bf16(xt); xpad <- batch-32-padded; x8 <- fp8
                for ki in range(KO):
                    nc.vector.tensor_copy(xbfs[ki][:], xts[ki][:])
                    src = xbfs[ki][:].rearrange("p (b s) -> p b s", s=S)
                    dst = xpads[ki][:].rearrange("p (b s) -> p b s", s=32)[:, :, 0:S]
                    nc.vector.tensor_copy(dst, src)
                    nc.vector.tensor_copy(x8p[ki // 2][:, ki % 2, :], xbfs[ki][:])

            # ================= layers =================
            with (
                tc.tile_pool(name="wpool", bufs=2) as wpool,
                tc.tile_pool(name="acts", bufs=1) as acts,
                tc.tile_pool(name="epool", bufs=9) as epool,
                tc.tile_pool(name="rzpool", bufs=2) as rzpool,
                tc.tile_pool(name="psB", bufs=5, space="PSUM") as psB,  # scores/Z/oT [128,384]
            ):
                qts = [acts.tile([128, T], BF16, tag=f"qt{o}", name=f"qt{o}") for o in range(8)]
                kts = [acts.tile([128, T], BF16, tag=f"kt{o}", name=f"kt{o}") for o in range(8)]
                vts = [acts.tile([128, D], BF16, tag=f"vt{g}", name=f"vt{g}") for g in range(8)]

                # DMA completion semaphores are monotonic per-queue counters:
                # the first matmul waits for EVERYTHING queued before its own
                # inputs.  So queue in exact first-consumption order: the x0
                # pair and qw pair each psum-chain step needs, interleaved.
                # Meanwhile the tensor engine runs warm-up matmuls on a
                # memset tile (no DMA dep) so it ramps to full clock instead
                # of idling through the cold DMA stream.
                warm = persist.tile([128, 512], BF16, name="warm")
                nc.vector.memset(warm[:], 0.001)
                for l in range(n_layers):
                    # fp8 q/k weights in ko-pair tiles for DoubleRow lhsT
                    qw_p = [wpool.tile([128, 2, D], FP8, tag=f"qw{j}", name=f"qw_{l}_{j}")
                            for j in range(KO // 2)]
                    kw_p = [wpool.tile([128, 2, D], FP8, tag=f"kw{j}", name=f"kw_{l}_{j}")
                            for j in range(KO // 2)]
                    vw_t = wpool.tile([128, KO, D], BF16, tag="vw")
                    qw_view = qwt_d[l].rearrange("(kj two p) o -> p kj two o", p=128, two=2)
                    kw_view = kwt_d[l].rearrange("(kj two p) o -> p kj two o", p=128, two=2)
                    if l == 0:
                        for j in range(KO // 2):
                            nc.sync.dma_start(xts[2 * j][:], x0_view[:, 2 * j, :])
                            nc.sync.dma_start(xts[2 * j + 1][:], x0_view[:, 2 * j + 1, :])
                            nc.sync.dma_start(qw_p[j][:], qw_view[:, j])
                        for j in range(KO // 2):
                            nc.sync.dma_start(kw_p[j][:], kw_view[:, j])
                        nc.sync.dma_start(qb_sb[:], qbs_d[:].rearrange(
                            "l (ko p) -> p l ko", p=128))
                        nc.sync.dma_start(kb_sb[:], kb_d[:].rearrange(
                            "l (ko p) -> p l ko", p=128))
                        nc.sync.dma_start(vb_sb[:], vbb_d[:].rearrange(
                            "l (ko p) -> p l ko", p=128))
                        recast()
                        wps = psA.tile([128, 512], F32, tag="proj", name="warm_ps")
                        for _ in range(110):
                            nc.tensor.matmul(wps[:], warm[:, 0:128], warm[:],
                                             start=True, stop=True)
                    else:
                        for j in range(KO // 2):
                            nc.sync.dma_start(qw_p[j][:], qw_view[:, j])
                            nc.sync.dma_start(kw_p[j][:], kw_view[:, j])
                    nc.sync.dma_start(vw_t[:], vwt_d[l].rearrange("(ko p) o -> p ko o", p=128))

                    # ---- Q, K projections (feature-major out, double-fp8) ----
                    for w_p, b_sb, dsts, sc in (
                        (qw_p, qb_sb, qts, SCALE / WS),
                        (kw_p, kb_sb, kts, 1.0 / WS),
                    ):
                        for oi in range(8):
                            bias_ap = b_sb[:, l, oi:oi + 1]
                            for t0 in (0, 384):
                                ps = psA.tile([128, 512], F32, tag="proj",
                                              name=f"p_{l}_{oi}_{t0}")
                                for kj in range(KO // 2):
                                    nc.tensor.matmul(
                                        ps[:, 0:384],
                                        w_p[kj][:, :, oi * 128:(oi + 1) * 128],
                                        x8p[kj][:, :, t0:t0 + 384],
                                        start=(kj == 0), stop=(kj == KO // 2 - 1),
                                        perf_mode=DR)
                                nc.scalar.activation(dsts[oi][:, t0:t0 + 384], ps[:, 0:384],
                                                     Ident, bias=bias_ap, scale=sc)

                    # ---- V projection (token-major, 32-padded batches) ----
                    for bg in range(8):
                        pv0 = psA.tile([128, 512], F32, tag="proj", name=f"pv0_{l}_{bg}")
                        pv1 = psA.tile([128, 512], F32, tag="proj", name=f"pv1_{l}_{bg}")
                        # no vb here: attn rows sum to 1, so o = attn@v0 + vb;
                        # vb is added straight into the residual xt instead
                        for ki in range(KO):
                            lhsT = xpads[ki][:, bg * 128:(bg + 1) * 128]
                            nc.tensor.matmul(pv0[:], lhsT, vw_t[:, ki, 0:512],
                                             start=(ki == 0), stop=(ki == KO - 1))
                            nc.tensor.matmul(pv1[:], lhsT, vw_t[:, ki, 512:1024],
                                             start=(ki == 0), stop=(ki == KO - 1))
                        for oc, pv in ((0, pv0), (1, pv1)):
                            nc.vector.tensor_copy(
                                vts[bg][:, oc * 512:(oc + 1) * 512], pv[:])

                    # ---- attention ----
                    # exp_t column layout: col(h) = (h%2)*192 + (h//2)*24
                    alv = 4 if do_attn is True else float(do_attn)
                    exp_ts = []

                    def z_stage(bg):
                        # issued one bg behind the scores matmuls so the
                        # in-order tensor queue never waits on scalar's exp;
                        # psum comes from psA (idle during the scores phase)
                        exp_t = exp_ts[bg]
                        z_ps = psA.tile([128, 512], F32, tag="proj",
                                        name=f"z_{l}_{bg}")
                        nc.tensor.matmul(
                            z_ps[0:120, 0:384], onesblk[0:120, :], exp_t[0:120, :],
                            start=True, stop=True, tile_position=(0, 0))
                        rz = rzpool.tile([128, 384], F32, tag="rz",
                                         name=f"rz_{l}_{bg}")
                        nc.vector.reciprocal_approx_fast(rz[0:120, :], z_ps[0:120, 0:384])
                        if alv >= 3:
                            # normalize on GpSimd: DVE is the busy engine in
                            # this window (V copies + reciprocals)
                            nc.gpsimd.tensor_tensor(exp_t[0:120, :], exp_t[0:120, :],
                                                    rz[0:120, :], Mult)

                    for bg in range(8 if alv >= 1 else 0):
                        # scores^T: even heads (kt/qt rows 0:64) -> row-group-0
                        # bank; odd heads (rows 64:128) -> row-group-64 bank.
                        sc_e = psB.tile([128, 192], F32, tag="p384", name=f"se_{l}_{bg}")
                        sc_o = psB.tile([128, 192], F32, tag="p384", name=f"so_{l}_{bg}")
                        for bi in range(4):
                            b = bg * 4 + bi
                            tcol = b * S
                            for hj in range(8):
                                for par, sc_ps in ((0, sc_e), (1, sc_o)):
                                    pb = par * 64
                                    nc.tensor.matmul(
                                        sc_ps[bi * 32:bi * 32 + S, hj * S:(hj + 1) * S],
                                        kts[hj][pb:pb + DH, tcol:tcol + S],
                                        qts[hj][pb:pb + DH, tcol:tcol + S],
                                        start=True, stop=True,
                                        tile_position=(pb, bi * 32))
                        # exp_t interleaved: head h=2j -> cols j*48, h=2j+1 ->
                        # cols j*48+24, so a head-pair is a contiguous 48-col
                        # block (lets AV pair 2 heads per matmul)
                        exp_t = epool.tile([128, 384], BF16, tag="expt", name=f"ex_{l}_{bg}")
                        exp_ts.append(exp_t)
                        e4 = exp_t[:].rearrange("p (j two s) -> p j two s", two=2, s=S)
                        nc.scalar.activation(e4[:, :, 0, :], sc_e[:].rearrange(
                            "p (j s) -> p j s", s=S), Exp)
                        nc.scalar.activation(e4[:, :, 1, :], sc_o[:].rearrange(
                            "p (j s) -> p j s", s=S), Exp)
                        if alv >= 2 and bg >= 1:
                            z_stage(bg - 1)
                    if alv >= 2 and alv < 4:
                        z_stage(7)

                    # AV: bank = (head pair hp, batch-slot class bi); the 16
                    # matmuls in a bank share row group bi*32; cols g*24.
                    # z_stage(7) is tucked between the first psum's matmuls so
                    # scalar's exp(7) has cover.
                    for hp in range(8 if alv >= 4 else 0):
                        for bi in range(4):
                            o_ps = psB.tile([128, 384], F32, tag="p384", name=f"o_{l}_{hp}_{bi}")
                            for g in range(8):
                                if hp == 0 and bi == 0 and g == 7:
                                    z_stage(7)
                                for hh in range(2):
                                    nc.tensor.matmul(
                                        o_ps[hh * 64:hh * 64 + DH, g * S:(g + 1) * S],
                                        vts[g][bi * 32:bi * 32 + S,
                                               (hp * 2 + hh) * DH:(hp * 2 + hh + 1) * DH],
                                        exp_ts[g][bi * 32:bi * 32 + S,
                                                  hp * 48 + hh * S:hp * 48 + (hh + 1) * S],
                                        start=True, stop=True,
                                        tile_position=(bi * 32, hh * 64))
                            # residual: b = g*4+bi -> xt cols g*96 + bi*24
                            xsl = xts[hp][:].rearrange(
                                "p (g f) -> p g f", f=96)[:, :, bi * S:(bi + 1) * S]
                            nc.vector.tensor_tensor(
                                xsl, xsl,
                                o_ps[:, 0:192].rearrange("p (g f) -> p g f", f=S), Add)
                        # head-pair residuals done for all batches: fold in vb
                        # (exact: softmax rows sum to 1) on the idle GpSimd,
                        # then refresh the fp8 copy (feeds next layer's q/k -
                        # straight from xt, off the scalar chain) and the
                        # bf16/padded copies on ScalarE
                        nc.gpsimd.tensor_scalar_add(xts[hp][:], xts[hp][:],
                                                    vb_sb[:, l, hp:hp + 1])
                        nc.vector.tensor_copy(x8p[hp // 2][:, hp % 2, :], xts[hp][:])
                        nc.scalar.copy(xbfs[hp][:], xts[hp][:])
                        srcp = xbfs[hp][:].rearrange("p (b s) -> p b s", s=S)
                        dstp = xpads[hp][:].rearrange("p (b s) -> p b s", s=32)[:, :, 0:S]
                        nc.scalar.copy(dstp, srcp)

            # ================= vocab head =================
            if n_layers == 0:
                for ki in range(KO):
                    nc.sync.dma_start(xts[ki][:], x0_view[:, ki, :])
            if n_layers == 0 or (do_attn is not True and float(do_attn) < 4):
                recast()
            CHUNK = 2048
            with (
                tc.tile_pool(name="owpool", bufs=3) as owpool,
                tc.tile_pool(name="lgpool", bufs=8) as lgpool,
                tc.tile_pool(name="psH", bufs=5, space="PSUM") as psH,
            ):
                for c0 in range(0, V if do_head else 0, CHUNK):
                    cw = min(CHUNK, V - c0)
                    owc = owpool.tile([128, KO, CHUNK], BF16, tag="ow", name=f"ow_{c0}")
                    for ki in range(KO):
                        nc.sync.dma_start(
                            owc[:, ki, 0:cw],
                            owt_d[:].rearrange("(ko p) v -> p ko v", p=128)[:, ki, c0:c0 + cw])
                    for sub0 in range(0, cw, 512):
                        sw = min(512, cw - sub0)
                        for tt in range(T // 128):
                            ps = psH.tile([128, 512], F32, tag="hps", name=f"h_{c0}_{sub0}_{tt}")
                            for ki in range(KO):
                                nc.tensor.matmul(
                                    ps[:, 0:sw],
                                    xbfs[ki][:, tt * 128:(tt + 1) * 128],
                                    owc[:, ki, sub0:sub0 + sw],
                                    start=(ki == 0), stop=(ki == KO - 1))
                            lg = lgpool.tile([128, 512], F32, tag="lg", name=f"lg_{c0}_{sub0}_{tt}")
                            nc.vector.tensor_copy(lg[:, 0:sw], ps[:, 0:sw])
                            nc.sync.dma_start(
                                out_d[tt * 128:(tt + 1) * 128, c0 + sub0:c0 + sub0 + sw],
                                lg[:, 0:sw])
    nc.finalize()
    return nc


def kernel(**inputs):
    inputs = {k: np.asarray(v) for k, v in inputs.items()}
    tok = inputs["inputs"]            # [S, B] int
    emb = inputs["token_embed"]       # [V, D] f32
    pe = inputs["pe"]                 # [27, 1, D] f32
    qw, qb = inputs["qw"], inputs["qb"]
    kw, kb = inputs["kw"], inputs["kb"]
    vw, vb = inputs["vw"], inputs["vb"]
    out_w, out_b = inputs["out_w"], inputs["out_b"]

    s_len = tok.shape[0]
    x0 = emb[tok] + pe[:s_len]        # [S, B, D] f32 (host: 0.006% of FLOPs)
    x0 = np.ascontiguousarray(x0, dtype=np.float32)

    bf = ml_dtypes.bfloat16
    f8 = ml_dtypes.float8_e4m3       # TRN float8e4: max normal +-240

    def to_f8(w):  # [L, D, D] -> transposed, scaled, e4m3
        wt = np.ascontiguousarray(w.transpose(0, 2, 1)).astype(np.float32)
        return np.clip(wt * WS, -240.0, 240.0).astype(f8)

    qwt = to_f8(qw)
    kwt = to_f8(kw)
    vwt = np.ascontiguousarray(vw.transpose(0, 2, 1)).astype(bf)
    owt = np.ascontiguousarray(out_w.T).astype(bf)
    qbs = (qb.astype(np.float32) * SCALE).astype(np.float32)
    kbf = kb.astype(np.float32)
    vbb = vb.astype(np.float32)

    if "nc" not in _CACHE:
        _CACHE["nc"] = _build_kernel()
    nc = _CACHE["nc"]

    shared = {
        "qwt": qwt, "kwt": kwt, "vwt": vwt,
        "qbs": qbs, "kb": kbf, "vbb": vbb,
        "owt": owt,
    }
    in_maps = []
    for c in range(NCORES):
        xc = x0[:, c * BL:(c + 1) * BL, :]            # [S, BL, D]
        x0t = np.ascontiguousarray(xc.transpose(2, 1, 0).reshape(D, T))
        in_maps.append({"x0t": x0t.astype(np.float32), **shared})

    res = run_bass_kernel_spmd(nc, in_maps, core_ids=list(range(NCORES)))
    _CACHE["last"] = res
    outs = [res.results[c]["out"] for c in range(NCORES)]   # each [T, V]
    full = np.stack(outs)                                    # [8, 768, V]
    full = full.reshape(NCORES, BL, S, V).transpose(2, 0, 1, 3).reshape(S, B, V)
    full = np.ascontiguousarray(full)
    if np.any(out_b):
        full += out_b.astype(np.float32)
    return full


# revision 20
# speedup vs baseline: 1.5167x; 1.5167x over previous
"""Trainium2 Bass kernel for a 12-layer attention-only decoder.

Model (see harness reference): S=24, B=256, D=1024, H=16 heads (dh=64),
L=12 layers, V=32000 vocab.  Per layer: q/k/v projections, softmax
attention (scale 1/sqrt(D), no mask applied), residual add.  Final vocab
head x @ out_w.T + out_b.

Sharding: data-parallel over batch - 8 cores x 32 batches each.
Embedding gather + positional-encoding add is done on host (0.006% of
FLOPs); everything else runs on device with fp32 accumulation.

Precision: Q/K projections run as double-fp8 (e4m3 weights x e4m3
activations, DoubleRow perf mode, 2x PE throughput) - softmax washes the
quantization noise out of the scores entirely (measured end-to-end rel
err identical to bf16).  V projection and the vocab head stay bf16 (v
errors hit the output linearly; fp8 there measures 6e-2 rel err).

On-device layout (per core, 768 tokens = 32 batches x 24 positions):
  - residual x kept feature-major: xT[d, t] as 8 chunk tiles [128, 768]
  - q/k projections produce feature-major qT/kT (stationary = w^T chunk)
  - v projection produces token-major v (stationary = xT chunk), padded
    so each batch sits at a 32-aligned partition base (24 rows used + 8
    pad) -> attention matmul operands satisfy the 32/64-alignment rules
  - scores^T[s', s] = matmul(lhsT=kT slice [64,24], rhs=qT slice [64,24])
  - exp via ScalarE (scale 1/32 folded into q), Z via ones-matmul issued
    one batch-group behind scores (keeps the in-order tensor queue from
    stalling on ScalarE), normalize on VectorE, AV: matmul(lhsT=v slice
    [24,64], rhs=attn^T [24,24]) -> o^T feature-major, accumulated
    straight into xT
  - vb folded into the residual after AV (exact: softmax rows sum to 1)
  - vocab head token-major: psum [128 tokens, 512 vocab] tiles, DMA out.
"""

import math

import numpy as np
import ml_dtypes

import concourse.bass as bass
import concourse.mybir as mybir
import concourse.tile as tile
from concourse import bacc
from concourse.bass_utils import run_bass_kernel_spmd

S, B, D, H, L, V = 24, 256, 1024, 16, 12, 32000
DH = D // H  # 64
NCORES = 8
BL = B // NCORES          # 32 local batches
T = BL * S                # 768 local tokens
KO = D // 128             # 8 contraction chunks
SCALE = 1.0 / math.sqrt(D)
WS = 256.0                # fp8 weight scale (power of 2; undone post-matmul)

F32 = mybir.dt.float32
BF16 = mybir.dt.bfloat16
FP8 = mybir.dt.float8e4

_CACHE = {}


def _build_kernel(n_layers=L, do_attn=True, do_head=True):
    nc = bacc.Bacc(None, target_bir_lowering=False)

    x0t_d = nc.dram_tensor("x0t", [D, T], F32, kind="ExternalInput")
    qwt_d = nc.dram_tensor("qwt", [L, D, D], FP8, kind="ExternalInput")
    kwt_d = nc.dram_tensor("kwt", [L, D, D], FP8, kind="ExternalInput")
    vwt_d = nc.dram_tensor("vwt", [L, D, D], BF16, kind="ExternalInput")
    qbs_d = nc.dram_tensor("qbs", [L, D], F32, kind="ExternalInput")
    kb_d = nc.dram_tensor("kb", [L, D], F32, kind="ExternalInput")
    vbb_d = nc.dram_tensor("vbb", [L, D], F32, kind="ExternalInput")
    owt_d = nc.dram_tensor("owt", [D, V], BF16, kind="ExternalInput")
    out_d = nc.dram_tensor("out", [T, V], F32, kind="ExternalOutput")

    Ident = mybir.ActivationFunctionType.Identity
    Exp = mybir.ActivationFunctionType.Exp
    Add = mybir.AluOpType.add
    Mult = mybir.AluOpType.mult
    DR = mybir.MatmulPerfMode.DoubleRow

    with tile.TileContext(nc) as tc:
        # zero all PSUM once: stale device PSUM may hold inf/NaN, which would
        # poison the block-diag Z matmul via 0*inf
        with tc.tile_pool(name="psinit", bufs=1, space="PSUM") as psi:
            for i in range(8):
                zb = psi.tile([128, 512], F32, name=f"zb_{i}", tag=f"zb_{i}")
                nc.vector.memset(zb[:], 0.0)

        with (
            tc.tile_pool(name="persist", bufs=1) as persist,
            tc.tile_pool(name="psA", bufs=3, space="PSUM") as psA,   # proj/head [128,512]
        ):
            # ---- persistent SBUF state ----
            # per-ko-chunk tiles: dependency tracking is tile-granular, so
            # separate tiles let layer-0 matmuls start on chunk 0 while the
            # rest of x0 is still in flight
            xts = [persist.tile([128, T], F32, name=f"xt{k}") for k in range(KO)]
            xbfs = [persist.tile([128, T], BF16, name=f"xbf{k}") for k in range(KO)]
            xpads = [persist.tile([128, BL * 32], BF16, name=f"xp{k}") for k in range(KO)]
            # fp8 x for the DoubleRow q/k projections, ko-pair layout
            x8p = [persist.tile([128, 2, T], FP8, name=f"x8p{j}") for j in range(KO // 2)]
            qb_sb = persist.tile([128, L, KO], F32, name="qb_sb")
            kb_sb = persist.tile([128, L, KO], F32, name="kb_sb")
            vb_sb = persist.tile([128, L, KO], F32, name="vb_sb")
            onesblk = persist.tile([128, 120], BF16, name="onesblk")

            x0_view = x0t_d[:].rearrange("(ko p) t -> p ko t", p=128)
            nc.vector.memset(onesblk[:], 0.0)
            for bi in range(4):
                nc.vector.memset(onesblk[bi * 32:bi * 32 + S, bi * 32:bi * 32 + S], 1.0)

            def recast():
                # xbf <- bf16(xt); xpad <- batch-32-padded; x8 <- fp8
                for ki in range(KO):
                    nc.vector.tensor_copy(xbfs[ki][:], xts[ki][:])
                    src = xbfs[ki][:].rearrange("p (b s) -> p b s", s=S)
                    dst = xpads[ki][:].rearrange("p (b s) -> p b s", s=32)[:, :, 0:S]
                    nc.vector.tensor_copy(dst, src)
                    nc.vector.tensor_copy(x8p[ki // 2][:, ki % 2, :], xbfs[ki][:])

            # ================= layers =================
            with (
                tc.tile_pool(name="wpool", bufs=2) as wpool,
                tc.tile_pool(name="acts", bufs=1) as acts,
                tc.tile_pool(name="epool", bufs=9) as epool,
                tc.tile_pool(name="rzpool", bufs=2) as rzpool,
                tc.tile_pool(name="psB", bufs=5, space="PSUM") as psB,  # scores/Z/oT [128,384]
            ):
                qts = [acts.tile([128, T], BF16, tag=f"qt{o}", name=f"qt{o}") for o in range(8)]
                kts = [acts.tile([128, T], BF16, tag=f"kt{o}", name=f"kt{o}") for o in range(8)]
                vts = [acts.tile([128, D], BF16, tag=f"vt{g}", name=f"vt{g}") for g in range(8)]

                # DMA completion semaphores are monotonic per-queue counters:
                # the first matmul waits for EVERYTHING queued before its own
                # inputs.  So queue in exact first-consumption order: the x0
                # pair and qw pair each psum-chain step needs, interleaved.
                # Meanwhile the tensor engine runs warm-up matmuls on a
                # memset tile (no DMA dep) so it ramps to full clock instead
                # of idling through the cold DMA stream.
                warm = persist.tile([128, 512], BF16, name="warm")
                nc.vector.memset(warm[:], 0.001)
                for l in range(n_layers):
                    # fp8 q/k weights in ko-pair tiles for DoubleRow lhsT
                    qw_p = [wpool.tile([128, 2, D], FP8, tag=f"qw{j}", name=f"qw_{l}_{j}")
                            for j in range(KO // 2)]
                    kw_p = [wpool.tile([128, 2, D], FP8, tag=f"kw{j}", name=f"kw_{l}_{j}")
                            for j in range(KO // 2)]
                    vw_t = wpool.tile([128, KO, D], BF16, tag="vw")
                    qw_view = qwt_d[l].rearrange("(kj two p) o -> p kj two o", p=128, two=2)
                    kw_view = kwt_d[l].rearrange("(kj two p) o -> p kj two o", p=128, two=2)
                    if l == 0:
                        for j in range(KO // 2):
                            nc.sync.dma_start(xts[2 * j][:], x0_view[:, 2 * j, :])
                            nc.sync.dma_start(xts[2 * j + 1][:], x0_view[:, 2 * j + 1, :])
                            nc.sync.dma_start(qw_p[j][:], qw_view[:, j])
                        for j in range(KO // 2):
                            nc.sync.dma_start(kw_p[j][:], kw_view[:, j])
                        nc.sync.dma_start(qb_sb[:], qbs_d[:].rearrange(
                            "l (ko p) -> p l ko", p=128))
                        nc.sync.dma_start(kb_sb[:], kb_d[:].rearrange(
                            "l (ko p) -> p l ko", p=128))
                        nc.sync.dma_start(vb_sb[:], vbb_d[:].rearrange(
                            "l (ko p) -> p l ko", p=128))
                        recast()
                        wps = psA.tile([128, 512], F32, tag="proj", name="warm_ps")
                        for _ in range(110):
                            nc.tensor.matmul(wps[:], warm[:, 0:128], warm[:],
                                             start=True, stop=True)
                    else:
                        for j in range(KO // 2):
                            nc.sync.dma_start(qw_p[j][:], qw_view[:, j])
                            nc.sync.dma_start(kw_p[j][:], kw_view[:, j])
                    nc.sync.dma_start(vw_t[:], vwt_d[l].rearrange("(ko p) o -> p ko o", p=128))

                    # ---- Q, K projections (feature-major out, double-fp8) ----
                    for w_p, b_sb, dsts, sc in (
                        (qw_p, qb_sb, qts, SCALE / WS),
                        (kw_p, kb_sb, kts, 1.0 / WS),
                    ):
                        for oi in range(8):
                            bias_ap = b_sb[:, l, oi:oi + 1]
                            for t0 in (0, 384):
                                ps = psA.tile([128, 512], F32, tag="proj",
                                              name=f"p_{l}_{oi}_{t0}")
                                for kj in range(KO // 2):
                                    nc.tensor.matmul(
                                        ps[:, 0:384],
                                        w_p[kj][:, :, oi * 128:(oi + 1) * 128],
                                        x8p[kj][:, :, t0:t0 + 384],
                                        start=(kj == 0), stop=(kj == KO // 2 - 1),
                                        perf_mode=DR)
                                nc.scalar.activation(dsts[oi][:, t0:t0 + 384], ps[:, 0:384],
                                                     Ident, bias=bias_ap, scale=sc)

                    # ---- V projection (token-major, 32-padded batches) ----
                    for bg in range(8):
                        pv0 = psA.tile([128, 512], F32, tag="proj", name=f"pv0_{l}_{bg}")
                        pv1 = psA.tile([128, 512], F32, tag="proj", name=f"pv1_{l}_{bg}")
                        # no vb here: attn rows sum to 1, so o = attn@v0 + vb;
                        # vb is added straight into the residual xt instead
                        for ki in range(KO):
                            lhsT = xpads[ki][:, bg * 128:(bg + 1) * 128]
                            nc.tensor.matmul(pv0[:], lhsT, vw_t[:, ki, 0:512],
                                             start=(ki == 0), stop=(ki == KO - 1))
                            nc.tensor.matmul(pv1[:], lhsT, vw_t[:, ki, 512:1024],
                                             start=(ki == 0), stop=(ki == KO - 1))
                        for oc, pv in ((0, pv0), (1, pv1)):
                            nc.vector.tensor_copy(
                                vts[bg][:, oc * 512:(oc + 1) * 512], pv[:])

                    # ---- attention ----
                    # exp_t column layout: col(h) = (h%2)*192 + (h//2)*24
                    alv = 4 if do_attn is True else float(do_attn)
                    exp_ts = []

                    def z_stage(bg):
                        # issued one bg behind the scores matmuls so the
                        # in-order tensor queue never waits on scalar's exp;
                        # psum comes from psA (idle during the scores phase)
                        exp_t = exp_ts[bg]
                        z_ps = psA.tile([128, 512], F32, tag="proj",
                                        name=f"z_{l}_{bg}")
                        nc.tensor.matmul(
                            z_ps[0:120, 0:384], onesblk[0:120, :], exp_t[0:120, :],
                            start=True, stop=True, tile_position=(0, 0))
                        rz = rzpool.tile([128, 384], F32, tag="rz",
                                         name=f"rz_{l}_{bg}")
                        nc.vector.reciprocal_approx_fast(rz[0:120, :], z_ps[0:120, 0:384])
                        if alv >= 3:
                            # normalize on GpSimd: DVE is the busy engine in
                            # this window (V copies + reciprocals)
                            nc.gpsimd.tensor_tensor(exp_t[0:120, :], exp_t[0:120, :],
                                                    rz[0:120, :], Mult)

                    for bg in range(8 if alv >= 1 else 0):
                        # scores^T: even heads (kt/qt rows 0:64) -> row-group-0
                        # bank; odd heads (rows 64:128) -> row-group-64 bank.
                        sc_e = psB.tile([128, 192], F32, tag="p384", name=f"se_{l}_{bg}")
                        sc_o = psB.tile([128, 192], F32, tag="p384", name=f"so_{l}_{bg}")
                        for bi in range(4):
                            b = bg * 4 + bi
                            tcol = b * S
                            for hj in range(8):
                                for par, sc_ps in ((0, sc_e), (1, sc_o)):
                                    pb = par * 64
                                    nc.tensor.matmul(
                                        sc_ps[bi * 32:bi * 32 + S, hj * S:(hj + 1) * S],
                                        kts[hj][pb:pb + DH, tcol:tcol + S],
                                        qts[hj][pb:pb + DH, tcol:tcol + S],
                                        start=True, stop=True,
                                        tile_position=(pb, bi * 32))
                        # exp_t interleaved: head h=2j -> cols j*48, h=2j+1 ->
                        # cols j*48+24, so a head-pair is a contiguous 48-col
                        # block (lets AV pair 2 heads per matmul)
                        exp_t = epool.tile([128, 384], BF16, tag="expt", name=f"ex_{l}_{bg}")
                        exp_ts.append(exp_t)
                        e4 = exp_t[:].rearrange("p (j two s) -> p j two s", two=2, s=S)
                        nc.scalar.activation(e4[:, :, 0, :], sc_e[:].rearrange(
                            "p (j s) -> p j s", s=S), Exp)
                        nc.scalar.activation(e4[:, :, 1, :], sc_o[:].rearrange(
                            "p (j s) -> p j s", s=S), Exp)
                        if alv >= 2 and bg >= 1:
                            z_stage(bg - 1)
                    if alv >= 2 and alv < 4:
                        z_stage(7)

                    # AV: bank = (head pair hp, batch-slot class bi); the 16
                    # matmuls in a bank share row group bi*32; cols g*24.
                    # z_stage(7) is tucked between the first psum's matmuls so
                    # scalar's exp(7) has cover.
                    for hp in range(8 if alv >= 4 else 0):
                        for bi in range(4):
                            o_ps = psB.tile([128, 384], F32, tag="p384", name=f"o_{l}_{hp}_{bi}")
                            for g in range(8):
                                if hp == 0 and bi == 0 and g == 7:
                                    z_stage(7)
                                for hh in range(2):
                                    nc.tensor.matmul(
                                        o_ps[hh * 64:hh * 64 + DH, g * S:(g + 1) * S],
                                        vts[g][bi * 32:bi * 32 + S,
                                               (hp * 2 + hh) * DH:(hp * 2 + hh + 1) * DH],
                                        exp_ts[g][bi * 32:bi * 32 + S,
                                                  hp * 48 + hh * S:hp * 48 + (hh + 1) * S],
                                        start=True, stop=True,
                                        tile_position=(bi * 32, hh * 64))
                            # residual: b = g*4+bi -> xt cols g*96 + bi*24
                            xsl = xts[hp][:].rearrange(
                                "p (g f) -> p g f", f=96)[:, :, bi * S:(bi + 1) * S]
                            nc.vector.tensor_tensor(
                                xsl, xsl,
                                o_ps[:, 0:192].rearrange("p (g f) -> p g f", f=S), Add)
                        # head-pair residuals done for all batches: fold in vb
                        # (exact: softmax rows sum to 1) on the idle GpSimd,
                        # then refresh the fp8 copy (feeds next layer's q/k -
                        # straight from xt, off the scalar chain) and the
                        # bf16/padded copies on ScalarE
                        nc.scalar.activation(xts[hp][:], xts[hp][:], Ident,
                                             bias=vb_sb[:, l, hp:hp + 1])
                        nc.vector.tensor_copy(x8p[hp // 2][:, hp % 2, :], xts[hp][:])
                        nc.scalar.copy(xbfs[hp][:], xts[hp][:])
                        srcp = xbfs[hp][:].rearrange("p (b s) -> p b s", s=S)
                        dstp = xpads[hp][:].rearrange("p (b s) -> p b s", s=32)[:, :, 0:S]
                        nc.scalar.copy(dstp, srcp)

            # ================= vocab head =================
            if n_layers == 0:
                for ki in range(KO):
                    nc.sync.dma_start(xts[ki][:], x0_view[:, ki, :])
            if n_layers == 0 or (do_attn is not True and float(do_attn) < 4):
                recast()
            CHUNK = 2048
            with (
                tc.tile_pool(name="owpool", bufs=3) as owpool,
                tc.tile_pool(name="lgpool", bufs=8) as lgpool,
                tc.tile_pool(name="psH", bufs=5, space="PSUM") as psH,
            ):
                for c0 in range(0, V if do_head else 0, CHUNK):
                    cw = min(CHUNK, V - c0)
                    owc = owpool.tile([128, KO, CHUNK], BF16, tag="ow", name=f"ow_{c0}")
                    for ki in range(KO):
                        nc.sync.dma_start(
                            owc[:, ki, 0:cw],
                            owt_d[:].rearrange("(ko p) v -> p ko v", p=128)[:, ki, c0:c0 + cw])
                    for sub0 in range(0, cw, 512):
                        sw = min(512, cw - sub0)
                        for tt in range(T // 128):
                            ps = psH.tile([128, 512], F32, tag="hps", name=f"h_{c0}_{sub0}_{tt}")
                            for ki in range(KO):
                                nc.tensor.matmul(
                                    ps[:, 0:sw],
                                    xbfs[ki][:, tt * 128:(tt + 1) * 128],
                                    owc[:, ki, sub0:sub0 + sw],
                                    start=(ki == 0), stop=(ki == KO - 1))
                            lg = lgpool.tile([128, 512], F32, tag="lg", name=f"lg_{c0}_{sub0}_{tt}")
                            nc.vector.tensor_copy(lg[:, 0:sw], ps[:, 0:sw])
                            nc.sync.dma_start(
                                out_d[tt * 128:(tt + 1) * 128, c0 + sub0:c0 + sub0 + sw],
                                lg[:, 0:sw])
    nc.finalize()
    return nc


def kernel(**inputs):
    inputs = {k: np.asarray(v) for k, v in inputs.items()}
    tok = inputs["inputs"]            # [S, B] int
    emb = inputs["token_embed"]       # [V, D] f32
    pe = inputs["pe"]                 # [27, 1, D] f32
    qw, qb = inputs["qw"], inputs["qb"]
    kw, kb = inputs["kw"], inputs["kb"]
    vw, vb = inputs["vw"], inputs["vb"]
    out_w, out_b = inputs["out_w"], inputs["out_b"]

    s_len = tok.shape[0]
    x0 = emb[tok] + pe[:s_len]        # [S, B, D] f32 (host: 0.006% of FLOPs)
    x0 = np.ascontiguousarray(x0, dtype=np.float32)

    bf = ml_dtypes.bfloat16
    f8 = ml_dtypes.float8_e4m3       # TRN float8e4: max normal +-240

    def to_f8(w):  # [L, D, D] -> transposed, scaled, e4m3
        wt = np.ascontiguousarray(w.transpose(0, 2, 1)).astype(np.float32)
        return np.clip(wt * WS, -240.0, 240.0).astype(f8)

    qwt = to_f8(qw)
    kwt = to_f8(kw)
    vwt = np.ascontiguousarray(vw.transpose(0, 2, 1)).astype(bf)
    owt = np.ascontiguousarray(out_w.T).astype(bf)
    qbs = (qb.astype(np.float32) * SCALE).astype(np.float32)
    kbf = kb.astype(np.float32)
    vbb = vb.astype(np.float32)

    if "nc" not in _CACHE:
        _CACHE["nc"] = _build_kernel()
    nc = _CACHE["nc"]

    shared = {
        "qwt": qwt, "kwt": kwt, "vwt": vwt,
        "qbs": qbs, "kb": kbf, "vbb": vbb,
        "owt": owt,
    }
    in_maps = []
    for c in range(NCORES):
        xc = x0[:, c * BL:(c + 1) * BL, :]            # [S, BL, D]
        x0t = np.ascontiguousarray(xc.transpose(2, 1, 0).reshape(D, T))
        in_maps.append({"x0t": x0t.astype(np.float32), **shared})

    res = run_bass_kernel_spmd(nc, in_maps, core_ids=list(range(NCORES)))
    _CACHE["last"] = res
    outs = [res.results[c]["out"] for c in range(NCORES)]   # each [T, V]
    full = np.stack(outs)                                    # [8, 768, V]
    full = full.reshape(NCORES, BL, S, V).transpose(2, 0, 1, 3).reshape(S, B, V)
    full = np.ascontiguousarray(full)
    if np.any(out_b):
        full += out_b.astype(np.float32)
    return full
